# revision 16
# baseline (speedup 1.0000x reference)
"""BinaryLinear (sign(x) @ sign(W).T + bias) on 8 trn2 NeuronCores.

Problem shapes (hardcoded): x [2, 4096, 4096] f32, weight [4096, 4096] f32,
bias [4096] f32. Output [2, 4096, 4096] f32.

Sharding: 2D grid — 4 token shards x 2 K-halves. Core (i, h) computes the
partial GEMM out_h[token_shard_i, :] = sign(x_i[:, kh]) @ sign(w[:, kh]).T
with M=2048 tokens, N=4096 outs, K=2048 contraction; the host adds the two
K-half partials (exact: integer partials |.| <= 2048 in f32). K-sharding
keeps per-core matmul work identical but cuts per-core input DMA from 64MB
to 48MB — the kernel's early phase is input-DMA-frontier limited, and
smaller full-K conversion blocks unlock matmul work sooner.

Host-side prep is layout-only (slice + transpose, values untouched): the PE
array contracts over the partition dim, so both operands are uploaded as
[K, *] f32. All math (sign, cast, GEMM, accumulation) runs on device.

Per-core pipeline. The kernel is DMA-frontier limited early on (64 MB of
f32 input at ~400 GB/s), so conversion is organized column-major: each x
m-block / w n-block covers the FULL contraction depth. An output unit
(128 m-rows x n-col) unlocks as soon as its two blocks are converted and
then runs all its 16 DoubleRow matmuls densely — no PSUM-bank residency
across the DMA stream. Block DMAs interleave x/w by bytes so unlocked
matmul work grows as fast as the DMA allows; the first w column is split
in two 256-wide halves so the PE starts ~20 us earlier.
  - weights:  DMA f32 [128k, kb, ncol] -> ACT Sign -> fp8e4 (+/-1 exact)
  - acts:     DMA f32 [128k, kb, mcol] -> DVE 2-op sign (t = max(x*1e30,-1)
              as bf16, then min(t,1) cast to fp8); exact for this data since
              min |v| ~ 7e-9 (guard in kernel()).
  - GEMM:     fp8 DoubleRow matmuls (K=256/step) accumulate f32 in PSUM;
              ACT copies PSUM->SBUF and issues the output DMA on the
              scalar HWDGE ring (input uses the sync ring).
Integer-valued sums (|.| <= 4096) are exact in fp32 => bit-exact output.
"""

import numpy as np

import concourse.bass as bass  # noqa: F401  (bass types used via APs)
import concourse.mybir as mybir
import concourse.tile as tile
from concourse import bacc
from concourse.bass_utils import run_bass_kernel_spmd

F32 = mybir.dt.float32
FP16 = mybir.dt.float16
BF16 = mybir.dt.bfloat16
FP8 = mybir.dt.float8e4

# Full problem
B, S, D_IN, D_OUT = 2, 4096, 4096, 4096
M_TOTAL = B * S  # 8192 tokens
# Shard grid: 4 token shards x 2 K-halves
TOK_SHARDS, K_SHARDS = 4, 2
N_CORES = TOK_SHARDS * K_SHARDS

# Per-core GEMM dims
M = M_TOTAL // TOK_SHARDS  # 2048
N = D_OUT                  # 4096
K = D_IN // K_SHARDS       # 2048

SIGN_SCALE = 1e30          # |v| >= 1e-30 maps to >= 1 pre-clamp


def build_nc(k=K, m=M, n=N, double_row=True):
    """Build + bacc-compile the per-core SPMD program.

    Inputs: xT [k, m] f32, wT [k, n] f32 (pre-transposed on host).
    Output: out [m, n] f32 = sign(xT).T @ sign(wT).
    """
    pair = 2 if double_row else 1
    kb = k // 128                    # 128-row k blocks
    ksup = kb // pair                # matmul k steps
    kc = min(8, kb)                  # k-blocks per 1MB-ish DMA chunk
    mblk = min(256, m)               # x conversion block (m columns)
    assert k % (128 * pair) == 0 and m % mblk == 0 and n % 512 == 0

    perf_mode = mybir.MatmulPerfMode.DoubleRow if double_row else None
    mult = mybir.AluOpType.mult
    amax = mybir.AluOpType.max
    amin = mybir.AluOpType.min

    # x blocks: first two 256-wide (early unlock), rest 512-wide (bigger DMA
    # descriptor runs).  w blocks: first 512 col split into two 256-wide
    # halves (early PE start), rest 512 wide.
    xblocks = [(s, mblk) for s in range(0, m, mblk)]
    wblocks = [(0, 256), (256, 256)] + [(s, 512) for s in range(512, n, 512)]

    # Interleave block DMAs by cumulative bytes (w leads).
    order = []
    wi = xi = wb = xb = 0
    while wi < len(wblocks) or xi < len(xblocks):
        if wi < len(wblocks) and (wb <= xb or xi >= len(xblocks)):
            order.append(("w", wi))
            wb += wblocks[wi][1]
            wi += 1
        else:
            order.append(("x", xi))
            xb += xblocks[xi][1]
            xi += 1

    # Output units: (m_tile, n_start, n_len). m-block MI covers m-tiles
    # MI*2, MI*2+1 (mblk=256). The first x-block keeps the two 256-wide
    # n-halves as separate units; later m-blocks use merged 512 columns.
    def unit_cols(bi):
        if bi == 0 and n >= 512:
            return [(0, 256), (256, 256)] + [(s, 512) for s in range(512, n, 512)]
        return [(s, 512) for s in range(0, n, 512)]

    nc = bacc.Bacc("TRN2", target_bir_lowering=False, debug=False)
    xT = nc.dram_tensor("xT", [k, m], F32, kind="ExternalInput").ap()
    wT = nc.dram_tensor("wT", [k, n], F32, kind="ExternalInput").ap()
    # fp16 partials: integer-valued, |.| <= k/2 <= 2048 — exact in fp16.
    out = nc.dram_tensor("out", [m, n], FP16, kind="ExternalOutput").ap()
    xTr = xT.rearrange("(b p) m -> p b m", p=128)  # [128, kb, m]
    wTr = wT.rearrange("(b p) n -> p b n", p=128)  # [128, kb, n]

    with tile.TileContext(nc) as tc:
        with (
            tc.tile_pool(name="conv", bufs=1) as conv,
            tc.tile_pool(name="stage", bufs=5) as stage,
            tc.tile_pool(name="tmp", bufs=2) as tmpp,
            tc.tile_pool(name="outp", bufs=4) as outp,
            tc.tile_pool(name="psum", bufs=8, space="PSUM") as psum,
        ):
            x_sb = conv.tile([128, kb, m], FP8)
            w_sb = conv.tile([128, kb, n], FP8)

            def conv_w(ns, nl):
                """ACT sign for w columns [ns, ns+nl), full K."""
                ckc = kc if nl == 256 else max(1, kc // 2)
                for c in range(0, kb, ckc):
                    st_w = stage.tile([128, ckc, nl], F32, tag="stw", bufs=4, name="st_w")
                    nc.sync.dma_start(st_w[:], wTr[:, c:c + ckc, ns:ns + nl])
                    nc.scalar.sign(w_sb[:, c:c + ckc, ns:ns + nl], st_w[:])

            def conv_x(ms, ml):
                """DVE sign for x columns [ms, ms+ml), full K."""
                ckc = kc if ml <= 256 else max(1, kc // 2)
                if ms == 0:
                    ckc = max(1, ckc // 2)
                for c in range(0, kb, ckc):
                    st_x = stage.tile([128, ckc, ml], F32, tag="stx", bufs=4, name="st_x")
                    nc.sync.dma_start(st_x[:], xTr[:, c:c + ckc, ms:ms + ml])
                    tmp = tmpp.tile([128, ckc, ml], BF16, tag="tmp", name="tmp")
                    nc.vector.tensor_scalar(tmp[:], st_x[:], SIGN_SCALE, -1.0,
                                            mult, amax)
                    nc.vector.tensor_scalar(
                        x_sb[:, c:c + ckc, ms:ms + ml], tmp[:], 1.0, None, amin)

            def emit_unit(mi, ns, nl, alt=False):
                """One output unit: m-tile mi x n columns [ns, ns+nl)."""
                pt = psum.tile([128, nl], F32, tag="pt", name=f"pt{mi}_{ns}",
                               padded_shape=[128, 512])
                for si in range(ksup):
                    nc.tensor.matmul(
                        pt[:],
                        x_sb[:, pair * si:pair * (si + 1), mi * 128:(mi + 1) * 128],
                        w_sb[:, pair * si:pair * (si + 1), ns:ns + nl],
                        start=(si == 0),
                        stop=(si == ksup - 1),
                        perf_mode=perf_mode,
                    )
                ot = outp.tile([128, 512], FP16, tag="ot", name="ot")
                nc.vector.tensor_copy(ot[:, :nl], pt[:])
                if alt:
                    # input stream has drained by the final wave; use sync ring
                    nc.sync.dma_start(out[mi * 128:(mi + 1) * 128, ns:ns + nl],
                                      ot[:, :nl])
                else:
                    nc.gpsimd.dma_start(out[mi * 128:(mi + 1) * 128, ns:ns + nl],
                                        ot[:, :nl])

            # Emit conversions + units in unlock order.
            done_w = [False] * len(wblocks)
            done_x = [False] * len(xblocks)
            emitted = set()

            def try_units(final):
                seq = 0
                for bi in range(len(xblocks)):
                    if not done_x[bi]:
                        continue
                    ms, ml = xblocks[bi]
                    for (ns, nl) in unit_cols(bi):
                        if (bi, ns) in emitted:
                            continue
                        need = [j for j, (ws, wl) in enumerate(wblocks)
                                if ws < ns + nl and ns < ws + wl]
                        if all(done_w[j] for j in need):
                            emitted.add((bi, ns))
                            for mi in range(ms // 128, (ms + ml) // 128):
                                emit_unit(mi, ns, nl, alt=final)
                                seq += 1

            for oi, (kind, idx) in enumerate(order):
                if kind == "w":
                    conv_w(*wblocks[idx])
                    done_w[idx] = True
                else:
                    conv_x(*xblocks[idx])
                    done_x[idx] = True
                try_units(final=(oi == len(order) - 1))
            assert len(emitted) == sum(len(unit_cols(bi)) for bi in range(len(xblocks)))

    nc.compile()
    return nc


def prepare_in_maps(x: np.ndarray, weight: np.ndarray):
    """Host-side shard + layout: both operands go up K-major."""
    x = np.ascontiguousarray(np.asarray(x, dtype=np.float32)).reshape(M_TOTAL, D_IN)
    weight = np.asarray(weight, dtype=np.float32)
    xT_shards = [
        np.ascontiguousarray(x[i * M:(i + 1) * M, :].T) for i in range(TOK_SHARDS)
    ]
    wT = np.ascontiguousarray(weight.T)  # [K_total, N]
    in_maps = []
    for c in range(N_CORES):
        i, h = divmod(c, K_SHARDS)
        in_maps.append({
            "xT": np.ascontiguousarray(xT_shards[i][h * K:(h + 1) * K, :]),
            "wT": np.ascontiguousarray(wT[h * K:(h + 1) * K, :]),
        })
    return in_maps


def assemble(results, bias: np.ndarray) -> np.ndarray:
    out = np.empty((M_TOTAL, D_OUT), dtype=np.float32)
    for i in range(TOK_SHARDS):
        np.add(results[i * K_SHARDS]["out"], results[i * K_SHARDS + 1]["out"],
               out=out[i * M:(i + 1) * M, :], dtype=np.float32, casting="unsafe")
    out += np.asarray(bias, dtype=np.float32)[None, :]
    return out.reshape(B, S, D_OUT)


def kernel(x: np.ndarray, weight: np.ndarray, bias: np.ndarray) -> np.ndarray:
    # DVE sign-via-clip needs |v|*SIGN_SCALE >= 1 for every nonzero input.
    for t in (x, weight):
        nz = np.abs(np.asarray(t, dtype=np.float32))
        assert nz[nz > 0].min() * SIGN_SCALE >= 1.0
    in_maps = prepare_in_maps(x, weight)
    nc = build_nc()
    res = run_bass_kernel_spmd(nc, in_maps, core_ids=list(range(N_CORES)))
    return assemble(res.results, bias)


# revision 17
# speedup vs baseline: 1.1000x; 1.1000x over previous
"""BinaryLinear (sign(x) @ sign(W).T + bias) on 8 trn2 NeuronCores.

Problem shapes (hardcoded): x [2, 4096, 4096] f32, weight [4096, 4096] f32,
bias [4096] f32. Output [2, 4096, 4096] f32.

Sharding: 2D grid — 4 token shards x 2 K-halves. Core (i, h) computes the
partial GEMM out_h[token_shard_i, :] = sign(x_i[:, kh]) @ sign(w[:, kh]).T
with M=2048 tokens, N=4096 outs, K=2048 contraction; the host adds the two
K-half partials (exact: integer partials |.| <= 2048 in f32). K-sharding
keeps per-core matmul work identical but cuts per-core input DMA from 64MB
to 48MB — the kernel's early phase is input-DMA-frontier limited, and
smaller full-K conversion blocks unlock matmul work sooner.

Host-side prep is layout-only (slice + transpose, values untouched): the PE
array contracts over the partition dim, so both operands are uploaded as
[K, *] f32. All math (sign, cast, GEMM, accumulation) runs on device.

Per-core pipeline. The kernel is DMA-frontier limited early on (64 MB of
f32 input at ~400 GB/s), so conversion is organized column-major: each x
m-block / w n-block covers the FULL contraction depth. An output unit
(128 m-rows x n-col) unlocks as soon as its two blocks are converted and
then runs all its 16 DoubleRow matmuls densely — no PSUM-bank residency
across the DMA stream. Block DMAs interleave x/w by bytes so unlocked
matmul work grows as fast as the DMA allows; the first w column is split
in two 256-wide halves so the PE starts ~20 us earlier.
  - weights:  DMA f32 [128k, kb, ncol] -> ACT Sign -> fp8e4 (+/-1 exact)
  - acts:     DMA f32 [128k, kb, mcol] -> DVE 2-op sign (t = max(x*1e30,-1)
              as bf16, then min(t,1) cast to fp8); exact for this data since
              min |v| ~ 7e-9 (guard in kernel()).
  - GEMM:     fp8 DoubleRow matmuls (K=256/step) accumulate f32 in PSUM;
              ACT copies PSUM->SBUF and issues the output DMA on the
              scalar HWDGE ring (input uses the sync ring).
Integer-valued sums (|.| <= 4096) are exact in fp32 => bit-exact output.
"""

import numpy as np

import concourse.bass as bass  # noqa: F401  (bass types used via APs)
import concourse.mybir as mybir
import concourse.tile as tile
from concourse import bacc
from concourse.bass_utils import run_bass_kernel_spmd

F32 = mybir.dt.float32
FP16 = mybir.dt.float16
BF16 = mybir.dt.bfloat16
FP8 = mybir.dt.float8e4

# Full problem
B, S, D_IN, D_OUT = 2, 4096, 4096, 4096
M_TOTAL = B * S  # 8192 tokens
# Shard grid: 4 token shards x 2 K-halves
TOK_SHARDS, K_SHARDS = 4, 2
N_CORES = TOK_SHARDS * K_SHARDS

# Per-core GEMM dims
M = M_TOTAL // TOK_SHARDS  # 2048
N = D_OUT                  # 4096
K = D_IN // K_SHARDS       # 2048

SIGN_SCALE = 1e30          # |v| >= 1e-30 maps to >= 1 pre-clamp


def build_nc(k=K, m=M, n=N, double_row=True):
    """Build + bacc-compile the per-core SPMD program.

    Inputs: xT [k, m] f32, wT [k, n] f32 (pre-transposed on host).
    Output: out [m, n] f32 = sign(xT).T @ sign(wT).
    """
    pair = 2 if double_row else 1
    kb = k // 128                    # 128-row k blocks
    ksup = kb // pair                # matmul k steps
    kc = min(8, kb)                  # k-blocks per 1MB-ish DMA chunk
    mblk = min(256, m)               # x conversion block (m columns)
    assert k % (128 * pair) == 0 and m % mblk == 0 and n % 512 == 0

    perf_mode = mybir.MatmulPerfMode.DoubleRow if double_row else None
    mult = mybir.AluOpType.mult
    amax = mybir.AluOpType.max
    amin = mybir.AluOpType.min

    # x blocks: first two 256-wide (early unlock), rest 512-wide (bigger DMA
    # descriptor runs).  w blocks: first 512 col split into two 256-wide
    # halves (early PE start), rest 512 wide.
    xblocks = [(s, mblk) for s in range(0, m, mblk)]
    wblocks = [(0, 256), (256, 256)] + [(s, 512) for s in range(512, n, 512)]

    # Interleave block DMAs by cumulative bytes (w leads).
    order = []
    wi = xi = wb = xb = 0
    while wi < len(wblocks) or xi < len(xblocks):
        if wi < len(wblocks) and (wb <= xb or xi >= len(xblocks)):
            order.append(("w", wi))
            wb += wblocks[wi][1]
            wi += 1
        else:
            order.append(("x", xi))
            xb += xblocks[xi][1]
            xi += 1

    # Output units: (m_tile, n_start, n_len). m-block MI covers m-tiles
    # MI*2, MI*2+1 (mblk=256). The first x-block keeps the two 256-wide
    # n-halves as separate units; later m-blocks use merged 512 columns.
    def unit_cols(bi):
        if bi == 0 and n >= 512:
            return [(0, 256), (256, 256)] + [(s, 512) for s in range(512, n, 512)]
        return [(s, 512) for s in range(0, n, 512)]

    nc = bacc.Bacc("TRN2", target_bir_lowering=False, debug=False)
    xT = nc.dram_tensor("xT", [k, m], F32, kind="ExternalInput").ap()
    wT = nc.dram_tensor("wT", [k, n], F32, kind="ExternalInput").ap()
    # fp16 partials: integer-valued, |.| <= k/2 <= 2048 — exact in fp16.
    out = nc.dram_tensor("out", [m, n], FP16, kind="ExternalOutput").ap()
    xTr = xT.rearrange("(b p) m -> p b m", p=128)  # [128, kb, m]
    wTr = wT.rearrange("(b p) n -> p b n", p=128)  # [128, kb, n]

    with tile.TileContext(nc) as tc:
        with (
            tc.tile_pool(name="conv", bufs=1) as conv,
            tc.tile_pool(name="stage", bufs=5) as stage,
            tc.tile_pool(name="tmp", bufs=2) as tmpp,
            tc.tile_pool(name="outp", bufs=4) as outp,
            tc.tile_pool(name="psum", bufs=8, space="PSUM") as psum,
        ):
            x_sb = conv.tile([128, kb, m], FP8)
            w_sb = conv.tile([128, kb, n], FP8)

            def conv_w(ns, nl):
                """ACT sign for w columns [ns, ns+nl), full K."""
                ckc = kc if nl == 256 else max(1, kc // 2)
                for c in range(0, kb, ckc):
                    st_w = stage.tile([128, ckc, nl], F32, tag="stw", bufs=4, name="st_w")
                    nc.sync.dma_start(st_w[:], wTr[:, c:c + ckc, ns:ns + nl])
                    nc.scalar.sign(w_sb[:, c:c + ckc, ns:ns + nl], st_w[:])

            def conv_x(ms, ml):
                """DVE sign for x columns [ms, ms+ml), full K."""
                ckc = kc if ml <= 256 else max(1, kc // 2)
                if ms == 0:
                    ckc = max(1, ckc // 2)
                for c in range(0, kb, ckc):
                    st_x = stage.tile([128, ckc, ml], F32, tag="stx", bufs=4, name="st_x")
                    nc.sync.dma_start(st_x[:], xTr[:, c:c + ckc, ms:ms + ml])
                    tmp = tmpp.tile([128, ckc, ml], BF16, tag="tmp", name="tmp")
                    nc.vector.tensor_scalar(tmp[:], st_x[:], SIGN_SCALE, -1.0,
                                            mult, amax)
                    nc.vector.tensor_scalar(
                        x_sb[:, c:c + ckc, ms:ms + ml], tmp[:], 1.0, None, amin)

            def emit_unit(mi, ns, nl, alt=False):
                """One output unit: m-tile mi x n columns [ns, ns+nl)."""
                pt = psum.tile([128, nl], F32, tag="pt", name=f"pt{mi}_{ns}",
                               padded_shape=[128, 512])
                for si in range(ksup):
                    nc.tensor.matmul(
                        pt[:],
                        x_sb[:, pair * si:pair * (si + 1), mi * 128:(mi + 1) * 128],
                        w_sb[:, pair * si:pair * (si + 1), ns:ns + nl],
                        start=(si == 0),
                        stop=(si == ksup - 1),
                        perf_mode=perf_mode,
                    )
                ot = outp.tile([128, 512], FP16, tag="ot", name="ot")
                nc.vector.tensor_copy(ot[:, :nl], pt[:])
                nc.gpsimd.dma_start(out[mi * 128:(mi + 1) * 128, ns:ns + nl],
                                    ot[:, :nl])

            # Emit conversions + units in unlock order.
            done_w = [False] * len(wblocks)
            done_x = [False] * len(xblocks)
            emitted = set()

            def try_units(final):
                seq = 0
                for bi in range(len(xblocks)):
                    if not done_x[bi]:
                        continue
                    ms, ml = xblocks[bi]
                    for (ns, nl) in unit_cols(bi):
                        if (bi, ns) in emitted:
                            continue
                        need = [j for j, (ws, wl) in enumerate(wblocks)
                                if ws < ns + nl and ns < ws + wl]
                        if all(done_w[j] for j in need):
                            emitted.add((bi, ns))
                            for mi in range(ms // 128, (ms + ml) // 128):
                                emit_unit(mi, ns, nl, alt=final)
                                seq += 1

            for oi, (kind, idx) in enumerate(order):
                if kind == "w":
                    conv_w(*wblocks[idx])
                    done_w[idx] = True
                else:
                    conv_x(*xblocks[idx])
                    done_x[idx] = True
                try_units(final=(oi == len(order) - 1))
            assert len(emitted) == sum(len(unit_cols(bi)) for bi in range(len(xblocks)))

    nc.compile()
    return nc


def prepare_in_maps(x: np.ndarray, weight: np.ndarray):
    """Host-side shard + layout: both operands go up K-major."""
    x = np.ascontiguousarray(np.asarray(x, dtype=np.float32)).reshape(M_TOTAL, D_IN)
    weight = np.asarray(weight, dtype=np.float32)
    xT_shards = [
        np.ascontiguousarray(x[i * M:(i + 1) * M, :].T) for i in range(TOK_SHARDS)
    ]
    wT = np.ascontiguousarray(weight.T)  # [K_total, N]
    in_maps = []
    for c in range(N_CORES):
        i, h = divmod(c, K_SHARDS)
        in_maps.append({
            "xT": np.ascontiguousarray(xT_shards[i][h * K:(h + 1) * K, :]),
            "wT": np.ascontiguousarray(wT[h * K:(h + 1) * K, :]),
        })
    return in_maps


def assemble(results, bias: np.ndarray) -> np.ndarray:
    out = np.empty((M_TOTAL, D_OUT), dtype=np.float32)
    for i in range(TOK_SHARDS):
        np.add(results[i * K_SHARDS]["out"], results[i * K_SHARDS + 1]["out"],
               out=out[i * M:(i + 1) * M, :], dtype=np.float32, casting="unsafe")
    out += np.asarray(bias, dtype=np.float32)[None, :]
    return out.reshape(B, S, D_OUT)


def kernel(x: np.ndarray, weight: np.ndarray, bias: np.ndarray) -> np.ndarray:
    # DVE sign-via-clip needs |v|*SIGN_SCALE >= 1 for every nonzero input.
    for t in (x, weight):
        nz = np.abs(np.asarray(t, dtype=np.float32))
        assert nz[nz > 0].min() * SIGN_SCALE >= 1.0
    in_maps = prepare_in_maps(x, weight)
    nc = build_nc()
    res = run_bass_kernel_spmd(nc, in_maps, core_ids=list(range(N_CORES)))
    return assemble(res.results, bias)
